# revision 8
# baseline (speedup 1.0000x reference)
"""Multi-head attention (B=8, N=1024, C=768, H=12, D=64) on 8 TRN2
NeuronCores, data-parallel over batch. Self-contained: builds a Bass/Tile
kernel per core, runs SPMD via run_bass_kernel_spmd, returns full output.

Per-core dataflow (all matmuls f32r, fp32 storage):
  x[1024,768] -> xT[c,n] (PE transpose)            wT=qkv_w.T[c,o], pwT=proj_w.T
  qT,kT [o,n] = wT.T @ xT   (+bias per-partition)  v [n,o] = xT.T @ wT (+bias)
  per head h: scoresT[m,n] = kh.T.T @ qh  (K=64)
              expT = exp(SCALE*scoresT)            (ACT, from PSUM)
              avT[d+1,n] = [vh|1].T @ expT         (row 64 = softmax denom)
              attn_outT[c,n] = avT[0:64] * bcast(1/denom)
  out[n,c'] = attn_outT.T @ pwT (+bias) -> DMA out
"""
import sys

sys.path.insert(0, "/opt/trn_rl_repo")

from contextlib import ExitStack

import numpy as np

import concourse.bass as bass
import concourse.mybir as mybir
import concourse.tile as tile
from concourse.bass_utils import run_bass_kernel_spmd
from concourse.masks import make_identity

DIM = 768
HEADS = 12
HD = 64
N = 1024
SCALE = HD ** -0.5
P = 128
NT = N // P          # 8 n-tiles
CT = DIM // P        # 6 c-tiles
F32 = mybir.dt.float32
F32R = mybir.dt.float32r
Exp = mybir.ActivationFunctionType.Exp

N_CORES = 8


def R(ap):
    return ap.bitcast(F32R)


def build_nc():
    nc = bass.Bass(trn_type="TRN2", target_bir_lowering=False, debug=False,
                   enable_asserts=False)
    x_d = nc.declare_dram_parameter("x", [N, DIM], F32, isOutput=False).ap()
    qkvw_d = nc.declare_dram_parameter("qkv_w", [3 * DIM, DIM], F32, isOutput=False).ap()
    qkvb_d = nc.declare_dram_parameter("qkv_b", [3 * DIM], F32, isOutput=False).ap()
    projw_d = nc.declare_dram_parameter("proj_w", [DIM, DIM], F32, isOutput=False).ap()
    projb_d = nc.declare_dram_parameter("proj_b", [DIM], F32, isOutput=False).ap()
    out_d = nc.declare_dram_parameter("out", [N, DIM], F32, isOutput=True).ap()

    with tile.TileContext(nc) as tc, ExitStack() as top:
        const = top.enter_context(tc.tile_pool(name="const", bufs=1))
        identity = const.tile([P, P], F32)
        make_identity(nc, identity[:])
        ones = const.tile([P, P], F32R)  # all-ones, f32r (rounded via copy below)

        bcol_qk = const.tile([P, 2 * CT], F32)  # column ot = qkv_b[ot*128:+128]
        nc.sync.dma_start(bcol_qk[:], qkvb_d[0:2 * DIM].rearrange("(o p) -> p o", p=P))

        # broadcast bias tiles for v and proj ([128, 768], same row repeated)
        vbias = const.tile([P, DIM], F32)
        pbias = const.tile([P, DIM], F32)
        with tc.tile_pool(name="brow_pool", bufs=1) as brow_pool, \
             tc.tile_pool(name="psum_bias", bufs=1, space="PSUM") as psum_bias:
            ones_f = brow_pool.tile([P, P], F32)
            nc.vector.memset(ones_f[:], 1.0)
            nc.vector.tensor_copy(ones[:], ones_f[:])
            b_row_f = brow_pool.tile([1, 3 * DIM], F32)
            nc.sync.dma_start(b_row_f[:], qkvb_d.unsqueeze(0))
            pb_row_f = brow_pool.tile([1, DIM], F32)
            nc.sync.dma_start(pb_row_f[:], projb_d.unsqueeze(0))
            b_row = brow_pool.tile([1, 3 * DIM], F32R)
            nc.vector.tensor_copy(b_row[:], b_row_f[:])
            pb_row = brow_pool.tile([1, DIM], F32R)
            nc.vector.tensor_copy(pb_row[:], pb_row_f[:])
            for dst, src_row, off in ((vbias, b_row, 2 * DIM), (pbias, pb_row, 0)):
                pt = psum_bias.tile([P, DIM], F32, tag="pbias", name="pbias")
                for o0, osz in ((0, 512), (512, 256)):
                    nc.tensor.matmul(pt[:, o0:o0 + osz], ones[0:1, :],
                                     src_row[0:1, off + o0:off + o0 + osz],
                                     start=True, stop=True)
                nc.vector.tensor_copy(dst[:], pt[:])

        # persistent activations
        qkvT = top.enter_context(tc.tile_pool(name="qkvT", bufs=1))
        qT = [qkvT.tile([P, N], F32R, tag=f"qT{i}", name=f"qT{i}") for i in range(CT)]
        kT = [qkvT.tile([P, N], F32R, tag=f"kT{i}", name=f"kT{i}") for i in range(CT)]
        v_aug = [qkvT.tile([P, HEADS * (HD + 1)], F32R, tag=f"v{i}", name=f"v{i}") for i in range(NT)]
        # ---- phase A: load x/qkv_w, transpose, qkv matmuls ----
        with tc.tile_pool(name="xw", bufs=1) as xw, \
             tc.tile_pool(name="stage", bufs=4) as stage, \
             tc.tile_pool(name="psum_t", bufs=3, space="PSUM") as psum_t, \
             tc.tile_pool(name="psum_qkv", bufs=2, space="PSUM") as psum_qkv:
            xT = [xw.tile([P, N], F32R, tag=f"xT{i}", name=f"xT{i}") for i in range(CT)]
            wT = [xw.tile([P, 3 * DIM], F32R, tag=f"wT{i}", name=f"wT{i}") for i in range(CT)]

            for nt in range(NT):
                st = stage.tile([P, DIM], F32, tag="stage")
                nc.sync.dma_start(st[:], x_d[nt * P:(nt + 1) * P, :])
                for ct in range(CT):
                    pt = psum_t.tile([P, P], F32, tag="pt")
                    nc.tensor.transpose(pt[:], st[:, ct * P:(ct + 1) * P], identity[:])
                    nc.vector.tensor_copy(xT[ct][:, nt * P:(nt + 1) * P], pt[:])
            for ot in range(3 * CT):
                st = stage.tile([P, DIM], F32, tag="stage")
                nc.sync.dma_start(st[:], qkvw_d[ot * P:(ot + 1) * P, :])
                for ct in range(CT):
                    pt = psum_t.tile([P, P], F32, tag="pt")
                    nc.tensor.transpose(pt[:], st[:, ct * P:(ct + 1) * P], identity[:])
                    nc.vector.tensor_copy(wT[ct][:, ot * P:(ot + 1) * P], pt[:])

            # q, k in [o, n] layout
            for ot in range(2 * CT):
                pq = psum_qkv.tile([P, N], F32, tag="pqk", name="pq")
                for nch in range(2):
                    for ct in range(CT):
                        nc.tensor.matmul(
                            pq[:, nch * 512:(nch + 1) * 512],
                            R(wT[ct][:, ot * P:(ot + 1) * P]),
                            R(xT[ct][:, nch * 512:(nch + 1) * 512]),
                            start=(ct == 0), stop=(ct == CT - 1))
                dst = qT[ot] if ot < CT else kT[ot - CT]
                nc.scalar.add(dst[:], pq[:], bcol_qk[:, ot:ot + 1])
            # v in [n, o] layout, interleaved 65-stride with ones columns
            for nt in range(NT):
                pv = psum_qkv.tile([P, DIM], F32, tag="pqk", name="pv")
                for o0, osz in ((0, 512), (512, 256)):
                    for ct in range(CT):
                        nc.tensor.matmul(
                            pv[:, o0:o0 + osz],
                            R(xT[ct][:, nt * P:(nt + 1) * P]),
                            wT[ct][:, 2 * DIM + o0:2 * DIM + o0 + osz],
                            start=(ct == 0), stop=(ct == CT - 1))
                va3 = v_aug[nt][:].rearrange("p (h e) -> p h e", e=HD + 1)
                nc.vector.tensor_copy(va3[:, :, HD:HD + 1],
                                      ones[:, 0:HEADS].unsqueeze(2))
                for h0, hn, o0 in ((0, 8, 0), (8, 4, 512)):
                    nc.vector.tensor_add(
                        va3[:, h0:h0 + hn, 0:HD],
                        pv[:, o0:o0 + hn * HD].rearrange("p (h e) -> p h e", e=HD),
                        vbias[:, o0:o0 + hn * HD].rearrange("p (h e) -> p h e", e=HD))

        # ---- phases B+C scope: attn_outT and proj weights ----
        aoT_pool = top.enter_context(tc.tile_pool(name="aoT", bufs=1))
        attn_outT = [aoT_pool.tile([P, N], F32R, tag=f"aoT{i}", name=f"aoT{i}") for i in range(CT)]
        pw_pool = top.enter_context(tc.tile_pool(name="pwT", bufs=1))
        pwT = [pw_pool.tile([P, DIM], F32R, tag=f"pwT{i}", name=f"pwT{i}") for i in range(CT)]

        # ---- phase B: proj_w transpose + attention ----
        with tc.tile_pool(name="stage2", bufs=2) as stage2, \
             tc.tile_pool(name="psum_t2", bufs=1, space="PSUM") as psum_t2, \
             tc.tile_pool(name="expp", bufs=1) as expp, \
             tc.tile_pool(name="small", bufs=2) as small, \
             tc.tile_pool(name="psum_s", bufs=2, space="PSUM") as psum_s_pool, \
             tc.tile_pool(name="psum_av", bufs=1, space="PSUM") as psum_av_pool, \
             tc.tile_pool(name="psum_bc", bufs=1, space="PSUM") as psum_bc_pool:
            for ct2 in range(CT):
                st = stage2.tile([P, DIM], F32, tag="stage2")
                nc.sync.dma_start(st[:], projw_d[ct2 * P:(ct2 + 1) * P, :])
                for ct in range(CT):
                    pt = psum_t2.tile([P, P], F32, tag="pt2")
                    nc.tensor.transpose(pt[:], st[:, ct * P:(ct + 1) * P], identity[:])
                    nc.vector.tensor_copy(pwT[ct][:, ct2 * P:(ct2 + 1) * P], pt[:])

            expT = [expp.tile([P, N], F32R, tag=f"expT{mt}", name=f"expT{mt}") for mt in range(NT)]
            for h in range(HEADS):
                t_i, t_off = h // 2, (h % 2) * HD
                qh = qT[t_i][t_off:t_off + HD, :]
                kh = kT[t_i][t_off:t_off + HD, :]
                for mt in range(NT):
                    ps = psum_s_pool.tile([P, N], F32, tag="ps")
                    for nch in range(2):
                        nc.tensor.matmul(
                            ps[:, nch * 512:(nch + 1) * 512],
                            R(kh[:, mt * P:(mt + 1) * P]),
                            R(qh[:, nch * 512:(nch + 1) * 512]),
                            start=True, stop=True)
                        nc.scalar.activation(
                            expT[mt][:, nch * 512:(nch + 1) * 512],
                            ps[:, nch * 512:(nch + 1) * 512], Exp, scale=SCALE)
                pav = psum_av_pool.tile([HD + 1, N], F32, tag="pav")
                for nch in range(2):
                    for mt in range(NT):
                        nc.tensor.matmul(
                            pav[:, nch * 512:(nch + 1) * 512],
                            R(v_aug[mt][:, h * (HD + 1):(h + 1) * (HD + 1)]),
                            R(expT[mt][:, nch * 512:(nch + 1) * 512]),
                            start=(mt == 0), stop=(mt == NT - 1))
                recip = small.tile([1, N], F32R, tag="recip")
                with nc.allow_low_precision(reason="f32r matmul input"):
                    nc.vector.reciprocal(recip[:], pav[HD:HD + 1, :])
                for nch in range(2):
                    pbc = psum_bc_pool.tile([HD, 512], F32, tag="pbc", name="pbc")
                    nc.tensor.matmul(
                        pbc[:], ones[0:1, 0:HD],
                        recip[0:1, nch * 512:(nch + 1) * 512],
                        start=True, stop=True)
                    bc = small.tile([HD, 512], F32, tag="bc", name="bc")
                    nc.vector.tensor_copy(bc[:], pbc[:])
                    nc.vector.tensor_mul(
                        attn_outT[t_i][t_off:t_off + HD, nch * 512:(nch + 1) * 512],
                        pav[0:HD, nch * 512:(nch + 1) * 512], bc[:])

        # ---- phase C: proj ----
        with tc.tile_pool(name="outp", bufs=3) as outp, \
             tc.tile_pool(name="psum_o", bufs=2, space="PSUM") as psum_o_pool:
            for nt in range(NT):
                po = psum_o_pool.tile([P, DIM], F32, tag="po")
                for o0, osz in ((0, 512), (512, 256)):
                    for ct in range(CT):
                        nc.tensor.matmul(
                            po[:, o0:o0 + osz],
                            R(attn_outT[ct][:, nt * P:(nt + 1) * P]),
                            pwT[ct][:, o0:o0 + osz],
                            start=(ct == 0), stop=(ct == CT - 1))
                ot_t = outp.tile([P, DIM], F32, tag="out")
                nc.vector.tensor_add(ot_t[:], po[:], pbias[:])
                nc.sync.dma_start(out_d[nt * P:(nt + 1) * P, :], ot_t[:])

    split_waits(nc)
    return nc


def split_waits(nc):
    """This walrus codegen supports one sync wait per instruction; move
    extra Tile-emitted waits onto EventSemaphore instructions inserted
    just before, in the same engine's program order."""
    n_split = 0
    for bb in nc.m.functions[0].blocks:
        insts = bb.instructions
        new_insts = []
        for inst in insts:
            si = inst.sync_info
            if si is not None and si.on_wait and len(si.on_wait) > 1:
                waits = list(si.on_wait)
                for w in waits[:-1]:
                    ev = mybir.InstEventSemaphore(name=f"{inst.name}-ws{n_split}")
                    ev.engine = inst.engine
                    ev.sync_info = mybir.SyncInfo(on_wait=[w], on_update=[])
                    new_insts.append(ev)
                    n_split += 1
                si.on_wait = [waits[-1]]
                inst.sync_info = si
            new_insts.append(inst)
        if len(new_insts) != len(insts):
            insts[:] = new_insts
    return n_split


_NC_CACHE = None


def get_nc():
    global _NC_CACHE
    if _NC_CACHE is None:
        _NC_CACHE = build_nc()
    return _NC_CACHE


def run(inputs, **kwargs):
    nc = get_nc()
    x = np.ascontiguousarray(inputs["x"], dtype=np.float32)
    shared = {
        "qkv_w": np.ascontiguousarray(inputs["qkv_w"], dtype=np.float32),
        "qkv_b": np.ascontiguousarray(inputs["qkv_b"], dtype=np.float32),
        "proj_w": np.ascontiguousarray(inputs["proj_w"], dtype=np.float32),
        "proj_b": np.ascontiguousarray(inputs["proj_b"], dtype=np.float32),
    }
    in_maps = [{"x": x[i], **shared} for i in range(N_CORES)]
    res = run_bass_kernel_spmd(nc, in_maps, core_ids=list(range(N_CORES)), **kwargs)
    out = np.stack([res.results[i]["out"] for i in range(N_CORES)], axis=0)
    return out, res


def kernel(x, qkv_w, qkv_b, proj_w, proj_b):
    out, _ = run({"x": x, "qkv_w": qkv_w, "qkv_b": qkv_b,
                  "proj_w": proj_w, "proj_b": proj_b})
    return out


# revision 9
# speedup vs baseline: 1.2686x; 1.2686x over previous
"""Multi-head attention (B=8, N=1024, C=768, H=12, D=64) on 8 TRN2
NeuronCores, data-parallel over batch. Self-contained: builds a Bass/Tile
kernel per core, runs SPMD via run_bass_kernel_spmd, returns full output.

Per-core dataflow:
  x[1024,768] -> xT[c,n] (PE transpose, f32)       wT=qkv_w.T[c,o], pwT=proj_w.T
  qkv matmuls in f32r (full-rate fp32-ish):
    q,k -> per-head bf16 tiles [128,1024], rows 0-63 = head data, 64-127 zero
           (K padded to 128: K=64 matmuls run at half rate on this PE)
    v   -> v_aug[n, 12*128] bf16: per head 64 v-cols + 64 ones-cols
  per head h (bf16 matmuls):
    scoresT[m,n] = k_pad[h][:,mslice].T @ q_pad[h]      (PSUM f32)
    expT[m,n] = exp(SCALE*scoresT)                      (ACT, bf16 out)
    pav[128,n] = v_aug[h-slice].T @ expT  — rows 0-63 attn@v, 64-127 the
           softmax denominator replicated 64x (ones-columns trick)
    bc = 1/pav[64:128]  (DVE reciprocal, full 64-partition op)
    attn_outT[c,n] = pav[0:64] * bc                     (f32r out)
  out[n,c'] = attn_outT.T @ pwT + bias (f32r) -> DMA out
"""
import sys

sys.path.insert(0, "/opt/trn_rl_repo")

from contextlib import ExitStack

import numpy as np

import concourse.bass as bass
import concourse.mybir as mybir
import concourse.tile as tile
from concourse.bass_utils import run_bass_kernel_spmd
from concourse.masks import make_identity

DIM = 768
HEADS = 12
HD = 64
N = 1024
SCALE = HD ** -0.5
P = 128
NT = N // P          # 8 n-tiles
CT = DIM // P        # 6 c-tiles
F32 = mybir.dt.float32
F32R = mybir.dt.float32r
BF16 = mybir.dt.bfloat16
Exp = mybir.ActivationFunctionType.Exp

N_CORES = 8


def build_nc():
    nc = bass.Bass(trn_type="TRN2", target_bir_lowering=False, debug=False,
                   enable_asserts=False)
    x_d = nc.declare_dram_parameter("x", [N, DIM], F32, isOutput=False).ap()
    qkvw_d = nc.declare_dram_parameter("qkv_w", [3 * DIM, DIM], F32, isOutput=False).ap()
    qkvb_d = nc.declare_dram_parameter("qkv_b", [3 * DIM], F32, isOutput=False).ap()
    projw_d = nc.declare_dram_parameter("proj_w", [DIM, DIM], F32, isOutput=False).ap()
    projb_d = nc.declare_dram_parameter("proj_b", [DIM], F32, isOutput=False).ap()
    out_d = nc.declare_dram_parameter("out", [N, DIM], F32, isOutput=True).ap()

    with tile.TileContext(nc) as tc, ExitStack() as top:
        const = top.enter_context(tc.tile_pool(name="const", bufs=1))
        identity = const.tile([P, P], F32)
        make_identity(nc, identity[:])
        ones = const.tile([P, P], F32R)  # all-ones, f32r (rounded via copy below)

        bcol_qk = const.tile([P, 2 * CT], F32)  # column ot = qkv_b[ot*128:+128]
        nc.sync.dma_start(bcol_qk[:], qkvb_d[0:2 * DIM].rearrange("(o p) -> p o", p=P))

        # broadcast bias tiles for v and proj ([128, 768], same row repeated)
        vbias = const.tile([P, DIM], F32)
        pbias = const.tile([P, DIM], F32)
        with tc.tile_pool(name="brow_pool", bufs=1) as brow_pool, \
             tc.tile_pool(name="psum_bias", bufs=1, space="PSUM") as psum_bias:
            ones_f = brow_pool.tile([P, P], F32)
            nc.vector.memset(ones_f[:], 1.0)
            nc.vector.tensor_copy(ones[:], ones_f[:])
            b_row_f = brow_pool.tile([1, 3 * DIM], F32)
            nc.sync.dma_start(b_row_f[:], qkvb_d.unsqueeze(0))
            pb_row_f = brow_pool.tile([1, DIM], F32)
            nc.sync.dma_start(pb_row_f[:], projb_d.unsqueeze(0))
            b_row = brow_pool.tile([1, 3 * DIM], F32R)
            nc.vector.tensor_copy(b_row[:], b_row_f[:])
            pb_row = brow_pool.tile([1, DIM], F32R)
            nc.vector.tensor_copy(pb_row[:], pb_row_f[:])
            for dst, src_row, off in ((vbias, b_row, 2 * DIM), (pbias, pb_row, 0)):
                pt = psum_bias.tile([P, DIM], F32, tag="pbias", name="pbias")
                for o0, osz in ((0, 512), (512, 256)):
                    nc.tensor.matmul(pt[:, o0:o0 + osz], ones[0:1, :],
                                     src_row[0:1, off + o0:off + o0 + osz],
                                     start=True, stop=True)
                nc.vector.tensor_copy(dst[:], pt[:])

        # persistent activations: padded per-head q/k (bf16), interleaved v_aug
        qkvT = top.enter_context(tc.tile_pool(name="qkvT", bufs=1))
        q_pad = [qkvT.tile([P, N], BF16, tag=f"qp{h}", name=f"qp{h}") for h in range(HEADS)]
        k_pad = [qkvT.tile([P, N], BF16, tag=f"kp{h}", name=f"kp{h}") for h in range(HEADS)]
        v_aug = [qkvT.tile([P, HEADS * P], BF16, tag=f"v{i}", name=f"v{i}") for i in range(NT)]
        for h in range(HEADS):
            nc.gpsimd.memset(q_pad[h][HD:P, :], 0.0)
            nc.gpsimd.memset(k_pad[h][HD:P, :], 0.0)
        for nt in range(NT):
            # ones-columns 64:128 per head (denominator replication trick)
            va3 = v_aug[nt][:].rearrange("p (h e) -> p h e", e=P)
            nc.gpsimd.memset(va3[:, :, HD:P], 1.0)

        # ---- phase A: load x/qkv_w, transpose, qkv matmuls ----
        with tc.tile_pool(name="xw", bufs=1) as xw, \
             tc.tile_pool(name="stage", bufs=4) as stage, \
             tc.tile_pool(name="psum_t", bufs=3, space="PSUM") as psum_t, \
             tc.tile_pool(name="psum_qkv", bufs=2, space="PSUM") as psum_qkv:
            xT = [xw.tile([P, N], F32R, tag=f"xT{i}", name=f"xT{i}") for i in range(CT)]
            wT = [xw.tile([P, 3 * DIM], F32R, tag=f"wT{i}", name=f"wT{i}") for i in range(CT)]

            for nt in range(NT):
                st = stage.tile([P, DIM], F32, tag="stage")
                nc.sync.dma_start(st[:], x_d[nt * P:(nt + 1) * P, :])
                for ct in range(CT):
                    pt = psum_t.tile([P, P], F32, tag="pt")
                    nc.tensor.transpose(pt[:], st[:, ct * P:(ct + 1) * P], identity[:])
                    nc.vector.tensor_copy(xT[ct][:, nt * P:(nt + 1) * P], pt[:])
            for ot in range(3 * CT):
                st = stage.tile([P, DIM], F32, tag="stage")
                nc.sync.dma_start(st[:], qkvw_d[ot * P:(ot + 1) * P, :])
                for ct in range(CT):
                    pt = psum_t.tile([P, P], F32, tag="pt")
                    nc.tensor.transpose(pt[:], st[:, ct * P:(ct + 1) * P], identity[:])
                    nc.vector.tensor_copy(wT[ct][:, ot * P:(ot + 1) * P], pt[:])

            # q, k in [o, n] layout -> split into padded per-head bf16 tiles
            for ot in range(2 * CT):
                pq = psum_qkv.tile([P, N], F32, tag="pqk", name="pq")
                for nch in range(2):
                    for ct in range(CT):
                        nc.tensor.matmul(
                            pq[:, nch * 512:(nch + 1) * 512],
                            wT[ct][:, ot * P:(ot + 1) * P],
                            xT[ct][:, nch * 512:(nch + 1) * 512],
                            start=(ct == 0), stop=(ct == CT - 1))
                if ot < CT:
                    ha, hb = 2 * ot, 2 * ot + 1
                    nc.scalar.add(q_pad[ha][0:HD, :], pq[0:HD, :],
                                  bcol_qk[0:HD, ot:ot + 1])
                    nc.scalar.add(q_pad[hb][0:HD, :], pq[HD:P, :],
                                  bcol_qk[HD:P, ot:ot + 1])
                else:
                    ko = ot - CT
                    ha, hb = 2 * ko, 2 * ko + 1
                    nc.vector.tensor_scalar_add(k_pad[ha][0:HD, :], pq[0:HD, :],
                                                bcol_qk[0:HD, ot:ot + 1])
                    nc.vector.tensor_scalar_add(k_pad[hb][0:HD, :], pq[HD:P, :],
                                                bcol_qk[HD:P, ot:ot + 1])
            # v in [n, o] layout, 128-stride interleave (64 v-cols + 64 ones)
            for nt in range(NT):
                pv = psum_qkv.tile([P, DIM], F32, tag="pqk", name="pv")
                for o0, osz in ((0, 512), (512, 256)):
                    for ct in range(CT):
                        nc.tensor.matmul(
                            pv[:, o0:o0 + osz],
                            xT[ct][:, nt * P:(nt + 1) * P],
                            wT[ct][:, 2 * DIM + o0:2 * DIM + o0 + osz],
                            start=(ct == 0), stop=(ct == CT - 1))
                va3 = v_aug[nt][:].rearrange("p (h e) -> p h e", e=P)
                for h0, hn, o0 in ((0, 8, 0), (8, 4, 512)):
                    nc.vector.tensor_add(
                        va3[:, h0:h0 + hn, 0:HD],
                        pv[:, o0:o0 + hn * HD].rearrange("p (h e) -> p h e", e=HD),
                        vbias[:, o0:o0 + hn * HD].rearrange("p (h e) -> p h e", e=HD))

        # ---- phases B+C scope: attn_outT and proj weights ----
        aoT_pool = top.enter_context(tc.tile_pool(name="aoT", bufs=1))
        attn_outT = [aoT_pool.tile([P, N], F32R, tag=f"aoT{i}", name=f"aoT{i}") for i in range(CT)]
        pw_pool = top.enter_context(tc.tile_pool(name="pwT", bufs=1))
        pwT = [pw_pool.tile([P, DIM], F32R, tag=f"pwT{i}", name=f"pwT{i}") for i in range(CT)]

        # ---- phase B: proj_w transpose + attention ----
        with tc.tile_pool(name="stage2", bufs=2) as stage2, \
             tc.tile_pool(name="psum_t2", bufs=2, space="PSUM") as psum_t2, \
             tc.tile_pool(name="expp", bufs=1) as expp, \
             tc.tile_pool(name="small", bufs=2) as small, \
             tc.tile_pool(name="psum_s", bufs=2, space="PSUM") as psum_s_pool, \
             tc.tile_pool(name="psum_av", bufs=1, space="PSUM") as psum_av_pool:
            for ct2 in range(CT):
                st = stage2.tile([P, DIM], F32, tag="stage2")
                nc.sync.dma_start(st[:], projw_d[ct2 * P:(ct2 + 1) * P, :])
                for ct in range(CT):
                    pt = psum_t2.tile([P, P], F32, tag="pt2")
                    nc.tensor.transpose(pt[:], st[:, ct * P:(ct + 1) * P], identity[:])
                    nc.vector.tensor_copy(pwT[ct][:, ct2 * P:(ct2 + 1) * P], pt[:])

            expT = [expp.tile([P, N], BF16, tag=f"expT{mt}", name=f"expT{mt}") for mt in range(NT)]
            for h in range(HEADS):
                t_i, t_off = h // 2, (h % 2) * HD
                for mt in range(NT):
                    ps = psum_s_pool.tile([P, N], F32, tag="ps")
                    for nch in range(2):
                        nc.tensor.matmul(
                            ps[:, nch * 512:(nch + 1) * 512],
                            k_pad[h][:, mt * P:(mt + 1) * P],
                            q_pad[h][:, nch * 512:(nch + 1) * 512],
                            start=True, stop=True)
                        nc.scalar.activation(
                            expT[mt][:, nch * 512:(nch + 1) * 512],
                            ps[:, nch * 512:(nch + 1) * 512], Exp, scale=SCALE)
                pav = psum_av_pool.tile([P, N], F32, tag="pav")
                for nch in range(2):
                    for mt in range(NT):
                        nc.tensor.matmul(
                            pav[:, nch * 512:(nch + 1) * 512],
                            v_aug[mt][:, h * P:(h + 1) * P],
                            expT[mt][:, nch * 512:(nch + 1) * 512],
                            start=(mt == 0), stop=(mt == NT - 1))
                bc = small.tile([HD, N], F32, tag="bc", name="bc")
                nc.vector.reciprocal(bc[:], pav[HD:P, :])
                nc.vector.tensor_mul(
                    attn_outT[t_i][t_off:t_off + HD, :], pav[0:HD, :], bc[:])

        # ---- phase C: proj ----
        with tc.tile_pool(name="outp", bufs=3) as outp, \
             tc.tile_pool(name="psum_o", bufs=2, space="PSUM") as psum_o_pool:
            for nt in range(NT):
                po = psum_o_pool.tile([P, DIM], F32, tag="po")
                for o0, osz in ((0, 512), (512, 256)):
                    for ct in range(CT):
                        nc.tensor.matmul(
                            po[:, o0:o0 + osz],
                            attn_outT[ct][:, nt * P:(nt + 1) * P],
                            pwT[ct][:, o0:o0 + osz],
                            start=(ct == 0), stop=(ct == CT - 1))
                ot_t = outp.tile([P, DIM], F32, tag="out")
                nc.vector.tensor_add(ot_t[:], po[:], pbias[:])
                nc.sync.dma_start(out_d[nt * P:(nt + 1) * P, :], ot_t[:])

    split_waits(nc)
    return nc


def split_waits(nc):
    """This walrus codegen supports one sync wait per instruction; move
    extra Tile-emitted waits onto EventSemaphore instructions inserted
    just before, in the same engine's program order."""
    n_split = 0
    for bb in nc.m.functions[0].blocks:
        insts = bb.instructions
        new_insts = []
        for inst in insts:
            si = inst.sync_info
            if si is not None and si.on_wait and len(si.on_wait) > 1:
                waits = list(si.on_wait)
                for w in waits[:-1]:
                    ev = mybir.InstEventSemaphore(name=f"{inst.name}-ws{n_split}")
                    ev.engine = inst.engine
                    ev.sync_info = mybir.SyncInfo(on_wait=[w], on_update=[])
                    new_insts.append(ev)
                    n_split += 1
                si.on_wait = [waits[-1]]
                inst.sync_info = si
            new_insts.append(inst)
        if len(new_insts) != len(insts):
            insts[:] = new_insts
    return n_split


_NC_CACHE = None


def get_nc():
    global _NC_CACHE
    if _NC_CACHE is None:
        _NC_CACHE = build_nc()
    return _NC_CACHE


def run(inputs, **kwargs):
    nc = get_nc()
    x = np.ascontiguousarray(inputs["x"], dtype=np.float32)
    shared = {
        "qkv_w": np.ascontiguousarray(inputs["qkv_w"], dtype=np.float32),
        "qkv_b": np.ascontiguousarray(inputs["qkv_b"], dtype=np.float32),
        "proj_w": np.ascontiguousarray(inputs["proj_w"], dtype=np.float32),
        "proj_b": np.ascontiguousarray(inputs["proj_b"], dtype=np.float32),
    }
    in_maps = [{"x": x[i], **shared} for i in range(N_CORES)]
    res = run_bass_kernel_spmd(nc, in_maps, core_ids=list(range(N_CORES)), **kwargs)
    out = np.stack([res.results[i]["out"] for i in range(N_CORES)], axis=0)
    return out, res


def kernel(x, qkv_w, qkv_b, proj_w, proj_b):
    out, _ = run({"x": x, "qkv_w": qkv_w, "qkv_b": qkv_b,
                  "proj_w": proj_w, "proj_b": proj_b})
    return out


# revision 11
# speedup vs baseline: 1.5510x; 1.2226x over previous
"""Multi-head attention (B=8, N=1024, C=768, H=12, D=64) on 8 TRN2
NeuronCores, data-parallel over batch. Self-contained: builds a Bass/Tile
kernel per core, runs SPMD via run_bass_kernel_spmd, returns full output.

Per-core dataflow:
  x[1024,768] -> xT[c,n] (PE transpose, f32)       wT=qkv_w.T[c,o], pwT=proj_w.T
  qkv matmuls in f32r (full-rate fp32-ish):
    q,k -> per-head bf16 tiles [128,1024], rows 0-63 = head data, 64-127 zero
           (K padded to 128: K=64 matmuls run at half rate on this PE)
    v   -> v_aug[n, 12*128] bf16: per head 64 v-cols + 64 ones-cols
  per head h (bf16 matmuls):
    scoresT[m,n] = k_pad[h][:,mslice].T @ q_pad[h]      (PSUM f32)
    expT[m,n] = exp(SCALE*scoresT)                      (ACT, bf16 out)
    pav[128,n] = v_aug[h-slice].T @ expT  — rows 0-63 attn@v, 64-127 the
           softmax denominator replicated 64x (ones-columns trick)
    bc = 1/pav[64:128]  (DVE reciprocal, full 64-partition op)
    attn_outT[c,n] = pav[0:64] * bc                     (f32r out)
  out[n,c'] = attn_outT.T @ pwT + bias (f32r) -> DMA out
"""
import sys

sys.path.insert(0, "/opt/trn_rl_repo")

from contextlib import ExitStack

import numpy as np

import concourse.bass as bass
import concourse.mybir as mybir
import concourse.tile as tile
from concourse.bass_utils import run_bass_kernel_spmd
from concourse.masks import make_identity

DIM = 768
HEADS = 12
HD = 64
N = 1024
SCALE = HD ** -0.5
P = 128
NT = N // P          # 8 n-tiles
CT = DIM // P        # 6 c-tiles
F32 = mybir.dt.float32
F32R = mybir.dt.float32r
BF16 = mybir.dt.bfloat16
Exp = mybir.ActivationFunctionType.Exp
Ln = mybir.ActivationFunctionType.Ln

N_CORES = 8


def build_nc():
    nc = bass.Bass(trn_type="TRN2", target_bir_lowering=False, debug=False,
                   enable_asserts=False)
    x_d = nc.declare_dram_parameter("x", [N, DIM], F32, isOutput=False).ap()
    qkvw_d = nc.declare_dram_parameter("qkv_w", [3 * DIM, DIM], F32, isOutput=False).ap()
    qkvb_d = nc.declare_dram_parameter("qkv_b", [3 * DIM], F32, isOutput=False).ap()
    projw_d = nc.declare_dram_parameter("proj_w", [DIM, DIM], F32, isOutput=False).ap()
    projb_d = nc.declare_dram_parameter("proj_b", [DIM], F32, isOutput=False).ap()
    out_d = nc.declare_dram_parameter("out", [N, DIM], F32, isOutput=True).ap()

    with tile.TileContext(nc) as tc, ExitStack() as top:
        const = top.enter_context(tc.tile_pool(name="const", bufs=1))
        identity = const.tile([P, P], F32)
        make_identity(nc, identity[:])
        ones = const.tile([P, P], F32R)  # all-ones, f32r (rounded via copy below)

        bcol_qk = const.tile([P, 2 * CT], F32)  # column ot = qkv_b[ot*128:+128]
        nc.sync.dma_start(bcol_qk[:], qkvb_d[0:2 * DIM].rearrange("(o p) -> p o", p=P))

        # broadcast bias tiles for v and proj ([128, 768], same row repeated)
        vbias = const.tile([P, DIM], F32)
        pbias = const.tile([P, DIM], F32)
        with tc.tile_pool(name="brow_pool", bufs=1) as brow_pool, \
             tc.tile_pool(name="psum_bias", bufs=1, space="PSUM") as psum_bias:
            ones_f = brow_pool.tile([P, P], F32)
            nc.vector.memset(ones_f[:], 1.0)
            nc.vector.tensor_copy(ones[:], ones_f[:])
            b_row_f = brow_pool.tile([1, 3 * DIM], F32)
            nc.sync.dma_start(b_row_f[:], qkvb_d.unsqueeze(0))
            pb_row_f = brow_pool.tile([1, DIM], F32)
            nc.sync.dma_start(pb_row_f[:], projb_d.unsqueeze(0))
            b_row = brow_pool.tile([1, 3 * DIM], F32R)
            nc.vector.tensor_copy(b_row[:], b_row_f[:])
            pb_row = brow_pool.tile([1, DIM], F32R)
            nc.vector.tensor_copy(pb_row[:], pb_row_f[:])
            for dst, src_row, off in ((vbias, b_row, 2 * DIM), (pbias, pb_row, 0)):
                pt = psum_bias.tile([P, DIM], F32, tag="pbias", name="pbias")
                for o0, osz in ((0, 512), (512, 256)):
                    nc.tensor.matmul(pt[:, o0:o0 + osz], ones[0:1, :],
                                     src_row[0:1, off + o0:off + o0 + osz],
                                     start=True, stop=True)
                nc.vector.tensor_copy(dst[:], pt[:])

        # persistent activations: padded per-head q/k (bf16), interleaved v_aug
        qkvT = top.enter_context(tc.tile_pool(name="qkvT", bufs=1))
        q_pad = [qkvT.tile([P, N], BF16, tag=f"qp{h}", name=f"qp{h}") for h in range(HEADS)]
        k_pad = [qkvT.tile([P, N], BF16, tag=f"kp{h}", name=f"kp{h}") for h in range(HEADS)]
        v_aug = [qkvT.tile([P, HEADS * P], BF16, tag=f"v{i}", name=f"v{i}") for i in range(NT)]
        for h in range(HEADS):
            nc.gpsimd.memset(q_pad[h][HD:P, :], 0.0)
            nc.gpsimd.memset(k_pad[h][HD:P, :], 0.0)
        for nt in range(NT):
            # ones-columns 64:128 per head (denominator replication trick)
            va3 = v_aug[nt][:].rearrange("p (h e) -> p h e", e=P)
            nc.gpsimd.memset(va3[:, :, HD:P], 1.0)

        # ---- phase A: load x/qkv_w, transpose, qkv matmuls ----
        with tc.tile_pool(name="xw", bufs=1) as xw, \
             tc.tile_pool(name="stage", bufs=4) as stage, \
             tc.tile_pool(name="psum_t", bufs=3, space="PSUM") as psum_t, \
             tc.tile_pool(name="psum_qkv", bufs=2, space="PSUM") as psum_qkv:
            xT = [xw.tile([P, N], F32R, tag=f"xT{i}", name=f"xT{i}") for i in range(CT)]
            wT = [xw.tile([P, 3 * DIM], F32R, tag=f"wT{i}", name=f"wT{i}") for i in range(CT)]

            for nt in range(NT):
                st = stage.tile([P, DIM], F32, tag="stage")
                nc.sync.dma_start(st[:], x_d[nt * P:(nt + 1) * P, :])
                for ct in range(CT):
                    pt = psum_t.tile([P, P], F32, tag="pt")
                    nc.tensor.transpose(pt[:], st[:, ct * P:(ct + 1) * P], identity[:])
                    nc.vector.tensor_copy(xT[ct][:, nt * P:(nt + 1) * P], pt[:])
            for ot in range(3 * CT):
                st = stage.tile([P, DIM], F32, tag="stage")
                nc.sync.dma_start(st[:], qkvw_d[ot * P:(ot + 1) * P, :])
                for ct in range(CT):
                    pt = psum_t.tile([P, P], F32, tag="pt")
                    nc.tensor.transpose(pt[:], st[:, ct * P:(ct + 1) * P], identity[:])
                    nc.vector.tensor_copy(wT[ct][:, ot * P:(ot + 1) * P], pt[:])

            # q, k in [o, n] layout -> split into padded per-head bf16 tiles
            for ot in range(2 * CT):
                pq = psum_qkv.tile([P, N], F32, tag="pqk", name="pq")
                for nch in range(2):
                    for ct in range(CT):
                        nc.tensor.matmul(
                            pq[:, nch * 512:(nch + 1) * 512],
                            wT[ct][:, ot * P:(ot + 1) * P],
                            xT[ct][:, nch * 512:(nch + 1) * 512],
                            start=(ct == 0), stop=(ct == CT - 1))
                if ot < CT:
                    ha, hb = 2 * ot, 2 * ot + 1
                    nc.scalar.add(q_pad[ha][0:HD, :], pq[0:HD, :],
                                  bcol_qk[0:HD, ot:ot + 1])
                    nc.scalar.add(q_pad[hb][0:HD, :], pq[HD:P, :],
                                  bcol_qk[HD:P, ot:ot + 1])
                else:
                    ko = ot - CT
                    ha, hb = 2 * ko, 2 * ko + 1
                    nc.vector.tensor_scalar_add(k_pad[ha][0:HD, :], pq[0:HD, :],
                                                bcol_qk[0:HD, ot:ot + 1])
                    nc.vector.tensor_scalar_add(k_pad[hb][0:HD, :], pq[HD:P, :],
                                                bcol_qk[HD:P, ot:ot + 1])
            # v in [n, o] layout, 128-stride interleave (64 v-cols + 64 ones)
            for nt in range(NT):
                pv = psum_qkv.tile([P, DIM], F32, tag="pqk", name="pv")
                for o0, osz in ((0, 512), (512, 256)):
                    for ct in range(CT):
                        nc.tensor.matmul(
                            pv[:, o0:o0 + osz],
                            xT[ct][:, nt * P:(nt + 1) * P],
                            wT[ct][:, 2 * DIM + o0:2 * DIM + o0 + osz],
                            start=(ct == 0), stop=(ct == CT - 1))
                va3 = v_aug[nt][:].rearrange("p (h e) -> p h e", e=P)
                for h0, hn, o0 in ((0, 8, 0), (8, 4, 512)):
                    nc.vector.tensor_add(
                        va3[:, h0:h0 + hn, 0:HD],
                        pv[:, o0:o0 + hn * HD].rearrange("p (h e) -> p h e", e=HD),
                        vbias[:, o0:o0 + hn * HD].rearrange("p (h e) -> p h e", e=HD))

        # ---- phases B+C scope: attn_outT and proj weights ----
        aoT_pool = top.enter_context(tc.tile_pool(name="aoT", bufs=1))
        attn_outT = [aoT_pool.tile([P, N], F32R, tag=f"aoT{i}", name=f"aoT{i}") for i in range(CT)]
        pw_pool = top.enter_context(tc.tile_pool(name="pwT", bufs=1))
        pwT = [pw_pool.tile([P, DIM], F32R, tag=f"pwT{i}", name=f"pwT{i}") for i in range(CT)]

        # ---- phase B: proj_w transpose + attention ----
        with tc.tile_pool(name="stage2", bufs=2) as stage2, \
             tc.tile_pool(name="psum_t2", bufs=1, space="PSUM") as psum_t2, \
             tc.tile_pool(name="expp", bufs=2) as expp, \
             tc.tile_pool(name="small", bufs=2) as small, \
             tc.tile_pool(name="psum_s", bufs=3, space="PSUM") as psum_s_pool, \
             tc.tile_pool(name="psum_av", bufs=2, space="PSUM") as psum_av_pool:
            for ct2 in range(CT):
                st = stage2.tile([P, DIM], F32, tag="stage2")
                nc.sync.dma_start(st[:], projw_d[ct2 * P:(ct2 + 1) * P, :])
                for ct in range(CT):
                    pt = psum_t2.tile([P, P], F32, tag="pt2")
                    nc.tensor.transpose(pt[:], st[:, ct * P:(ct + 1) * P], identity[:])
                    nc.vector.tensor_copy(pwT[ct][:, ct2 * P:(ct2 + 1) * P], pt[:])

            for h in range(HEADS):
                t_i, t_off = h // 2, (h % 2) * HD
                expT = [expp.tile([P, N], BF16, tag=f"expT{mt}", name=f"expT{mt}_{h}")
                        for mt in range(NT)]
                for mt in range(NT):
                    for nch in range(2):
                        ps = psum_s_pool.tile([P, 512], F32, tag="ps", name="ps")
                        nc.tensor.matmul(
                            ps[:],
                            k_pad[h][:, mt * P:(mt + 1) * P],
                            q_pad[h][:, nch * 512:(nch + 1) * 512],
                            start=True, stop=True)
                        nc.scalar.activation(
                            expT[mt][:, nch * 512:(nch + 1) * 512],
                            ps[:], Exp, scale=SCALE)
                pav = psum_av_pool.tile([P, N], F32, tag="pav")
                for nch in range(2):
                    for mt in range(NT):
                        nc.tensor.matmul(
                            pav[:, nch * 512:(nch + 1) * 512],
                            v_aug[mt][:, h * P:(h + 1) * P],
                            expT[mt][:, nch * 512:(nch + 1) * 512],
                            start=(mt == 0), stop=(mt == NT - 1))
                # 1/D as exp(-ln D) on ACT (custom-DVE recip ops don't
                # compile here; DVE InstReciprocal is ~6.4ns/elem/lane)
                lnd = small.tile([HD, N], F32, tag="lnd", name="lnd")
                nc.scalar.activation(lnd[:], pav[HD:P, :], Ln)
                bc = small.tile([HD, N], F32, tag="bc", name="bc")
                nc.scalar.activation(bc[:], lnd[:], Exp, scale=-1.0)
                nc.vector.tensor_mul(
                    attn_outT[t_i][t_off:t_off + HD, :], pav[0:HD, :], bc[:])

        # ---- phase C: proj ----
        with tc.tile_pool(name="outp", bufs=3) as outp, \
             tc.tile_pool(name="psum_o", bufs=2, space="PSUM") as psum_o_pool:
            for nt in range(NT):
                po = psum_o_pool.tile([P, DIM], F32, tag="po")
                for o0, osz in ((0, 512), (512, 256)):
                    for ct in range(CT):
                        nc.tensor.matmul(
                            po[:, o0:o0 + osz],
                            attn_outT[ct][:, nt * P:(nt + 1) * P],
                            pwT[ct][:, o0:o0 + osz],
                            start=(ct == 0), stop=(ct == CT - 1))
                ot_t = outp.tile([P, DIM], F32, tag="out")
                nc.vector.tensor_add(ot_t[:], po[:], pbias[:])
                nc.sync.dma_start(out_d[nt * P:(nt + 1) * P, :], ot_t[:])

    split_waits(nc)
    return nc


def split_waits(nc):
    """This walrus codegen supports one sync wait per instruction; move
    extra Tile-emitted waits onto EventSemaphore instructions inserted
    just before, in the same engine's program order."""
    n_split = 0
    for bb in nc.m.functions[0].blocks:
        insts = bb.instructions
        new_insts = []
        for inst in insts:
            si = inst.sync_info
            if si is not None and si.on_wait and len(si.on_wait) > 1:
                waits = list(si.on_wait)
                for w in waits[:-1]:
                    ev = mybir.InstEventSemaphore(name=f"{inst.name}-ws{n_split}")
                    ev.engine = inst.engine
                    ev.sync_info = mybir.SyncInfo(on_wait=[w], on_update=[])
                    new_insts.append(ev)
                    n_split += 1
                si.on_wait = [waits[-1]]
                inst.sync_info = si
            new_insts.append(inst)
        if len(new_insts) != len(insts):
            insts[:] = new_insts
    return n_split


_NC_CACHE = None


def get_nc():
    global _NC_CACHE
    if _NC_CACHE is None:
        _NC_CACHE = build_nc()
    return _NC_CACHE


def run(inputs, **kwargs):
    nc = get_nc()
    x = np.ascontiguousarray(inputs["x"], dtype=np.float32)
    shared = {
        "qkv_w": np.ascontiguousarray(inputs["qkv_w"], dtype=np.float32),
        "qkv_b": np.ascontiguousarray(inputs["qkv_b"], dtype=np.float32),
        "proj_w": np.ascontiguousarray(inputs["proj_w"], dtype=np.float32),
        "proj_b": np.ascontiguousarray(inputs["proj_b"], dtype=np.float32),
    }
    in_maps = [{"x": x[i], **shared} for i in range(N_CORES)]
    res = run_bass_kernel_spmd(nc, in_maps, core_ids=list(range(N_CORES)), **kwargs)
    out = np.stack([res.results[i]["out"] for i in range(N_CORES)], axis=0)
    return out, res


def kernel(x, qkv_w, qkv_b, proj_w, proj_b):
    out, _ = run({"x": x, "qkv_w": qkv_w, "qkv_b": qkv_b,
                  "proj_w": proj_w, "proj_b": proj_b})
    return out
